# revision 1
# baseline (speedup 1.0000x reference)
"""Trainium2 Bass kernel for nn_DrugGraphEmbedding (2-layer GCN over drug graphs).

Strategy: data-parallel over the batch axis across 8 NeuronCores (4 graphs per
core).  Per core:
  phase 1: stream the [4,16,256,256] embedding shard from HBM (both HWDGE
           rings), accumulate the pathway sum on the vector engine.
  phase 1b: gather-by-matmul of node features (gid one-hots pre-scaled with
           dis/P on the host), W1 matmul, PE transposes -> node-major scaled
           h1 table shard -> AllGather across the 8 cores.
  aggregation: dense-adjacency matmul on the tensor engine.  The host builds
           A [4096, 512] (0/1 with edge multiplicity, self-loops included) per
           core; agg^T[h, v] = sum_u hs^T[h, u] * A[u, v] runs as 64
           accumulating 128x128 @ 128x512 matmuls per layer.  The GCN
           symmetric norm factors into per-node dis[] scalings applied on the
           vector engine (free axis) so A stays exactly 0/1.  This avoids all
           per-edge DMA gathers (SWDGE descriptor generation is the
           bottleneck for those).
  layer 2 chains in the transposed layout through W2, then a dis-weighted
           pooling reduction produces the per-graph mean.
All compute is fp32.
"""

import os
import numpy as np

# ---- problem constants (hardcoded per contest rules) ----
B, P, S, D = 32, 16, 256, 256
NG = 128
N = B * NG            # 4096 nodes
E = 65536
H = 256
M = 8                 # cores
GPC = B // M          # 4 graphs per core
NPC = GPC * NG        # 512 nodes per core
NT = N // 128         # 32 node tiles

_CACHE = {}


# --------------------------------------------------------------------------
# host-side preprocessing (sharding + index prep)
# --------------------------------------------------------------------------
def _host_prep(global_ids, edge_index):
    gid = np.asarray(global_ids).astype(np.int64)
    ei = np.asarray(edge_index).astype(np.int64)
    src = np.concatenate([ei[0], np.arange(N, dtype=np.int64)])
    dst = np.concatenate([ei[1], np.arange(N, dtype=np.int64)])
    deg = np.bincount(dst, minlength=N).astype(np.float32)
    dis = (1.0 / np.sqrt(deg)).astype(np.float32)

    per_core = []
    for c in range(M):
        lo, hi = NPC * c, NPC * (c + 1)
        m = (dst >= lo) & (dst < hi)
        es, ed = src[m], dst[m] - lo
        # dense adjacency, rhs layout [src%128, (src//128, dst_local)];
        # np.add.at accumulates duplicate edges
        adj = np.zeros((128, NT, NPC), dtype=np.float32)
        np.add.at(adj, (es % 128, es // 128, ed), 1.0)
        # gid one-hot, scaled by dis/P:  [s_part, (g, s_half, i)]
        ohnx = np.zeros((128, GPC * 2 * 128), dtype=np.float32)
        for g in range(GPC):
            gnodes = lo + 128 * g + np.arange(NG)
            s = gid[GPC * c + g]
            ohnx[s % 128, (2 * g + (s // 128)) * 128 + np.arange(NG)] = (
                dis[gnodes] / P
            )
        dis_c = dis[lo:hi]
        disTr = np.ascontiguousarray(np.broadcast_to(dis_c, (128, NPC)))
        pwTr = np.ascontiguousarray(disTr / NG)
        per_core.append(dict(
            adj=np.ascontiguousarray(adj.reshape(128, NT * NPC)),
            ohnx=ohnx, disTr=disTr, pwTr=pwTr,
        ))
    return per_core


# --------------------------------------------------------------------------
# the Bass program (one SPMD program for all 8 cores)
# --------------------------------------------------------------------------
def _build_program(loop_r=None, loop_segs=(0, 1, 2)):
    """loop_r=None -> production program (each phase once).
    loop_r=R -> benchmark variant: each collective-free phase segment wrapped
    in a hardware For_i loop executing R times (phases are idempotent, so the
    output stays correct); used to measure device time via wall-clock slope."""
    import contextlib
    import concourse.bacc as bacc
    import concourse.tile as tile
    import concourse.mybir as mybir
    from concourse.bass import _add_dep_helper
    from concourse.masks import make_identity

    f32 = mybir.dt.float32
    AF = mybir.ActivationFunctionType
    ADD = mybir.AluOpType.add
    MUL = mybir.AluOpType.mult

    nc = bacc.Bacc("TRN2", target_bir_lowering=False, debug=False, num_devices=M)

    emb_t = nc.dram_tensor("emb", [GPC, P, S, D], f32, kind="ExternalInput")
    ohnx_t = nc.dram_tensor("ohnx", [128, GPC * 2 * 128], f32, kind="ExternalInput")
    w1_t = nc.dram_tensor("w1", [128, 512], f32, kind="ExternalInput")
    w2_t = nc.dram_tensor("w2", [128, 512], f32, kind="ExternalInput")
    adj_t = nc.dram_tensor("adj", [128, NT * NPC], f32, kind="ExternalInput")
    disTr_t = nc.dram_tensor("disTr", [128, NPC], f32, kind="ExternalInput")
    pwTr_t = nc.dram_tensor("pwTr", [128, NPC], f32, kind="ExternalInput")
    b1c_t = nc.dram_tensor("b1c", [128, 2], f32, kind="ExternalInput")
    b2c_t = nc.dram_tensor("b2c", [128, 2], f32, kind="ExternalInput")
    out_t = nc.dram_tensor("out", [GPC, H], f32, kind="ExternalOutput")

    cc1_in = nc.dram_tensor("cc1_in", [NPC, 256], f32, kind="Internal")
    cc2_in = nc.dram_tensor("cc2_in", [NPC, 256], f32, kind="Internal")
    table1 = nc.dram_tensor("table1", [N, 256], f32, kind="Internal",
                            addr_space="Shared")
    table2 = nc.dram_tensor("table2", [N, 256], f32, kind="Internal",
                            addr_space="Shared")
    RG = [list(range(M))]

    with tile.TileContext(nc) as tc:
        with (
            tc.tile_pool(name="const", bufs=1) as cpool,
            tc.tile_pool(name="stream", bufs=4) as spool,
            tc.tile_pool(name="work", bufs=1) as wpool,
            tc.tile_pool(name="hst", bufs=1) as hpool,
            tc.tile_pool(name="psum", bufs=4, space="PSUM") as ppool,
            tc.tile_pool(name="psmall", bufs=2, space="PSUM") as pspool,
        ):
            _seg_counter = [0]

            def seg():
                sid = _seg_counter[0]
                _seg_counter[0] += 1
                if loop_r is None or sid not in loop_segs:
                    return contextlib.nullcontext()
                return tc.For_i(0, loop_r, 1)

            # ---- constants in ----
            ohnx_sb = cpool.tile([128, GPC * 2 * 128], f32, name="ohnx_sb")
            w1_sb = cpool.tile([128, 512], f32, name="w1_sb")
            w2_sb = cpool.tile([128, 512], f32, name="w2_sb")
            adj_sb = cpool.tile([128, NT * NPC], f32, name="adj_sb")
            disTr_sb = cpool.tile([128, NPC], f32, name="disTr_sb")
            pwTr_sb = cpool.tile([128, NPC], f32, name="pwTr_sb")
            b1c_sb = cpool.tile([128, 2], f32, name="b1c_sb")
            b2c_sb = cpool.tile([128, 2], f32, name="b2c_sb")
            ident = cpool.tile([128, 128], f32, name="ident")

            nc.sync.dma_start(ohnx_sb[:], ohnx_t[:])
            nc.sync.dma_start(w1_sb[:], w1_t[:])
            nc.sync.dma_start(w2_sb[:], w2_t[:])
            nc.scalar.dma_start(disTr_sb[:], disTr_t[:])
            nc.scalar.dma_start(pwTr_sb[:], pwTr_t[:])
            nc.scalar.dma_start(b1c_sb[:], b1c_t[:])
            nc.scalar.dma_start(b2c_sb[:], b2c_t[:])
            make_identity(nc, ident[:])

            with seg():
                # ---- phase 1: pathway-sum accumulate  acc[s, (sh g d)] ----
                acc = cpool.tile([128, 2048], f32, name="acc")
                last_stream = None
                for p in range(P):
                    for sh in range(2):
                        pt = spool.tile([128, 1024], f32, name="pt")
                        eng = nc.sync if sh == 0 else nc.scalar
                        last_stream = eng.dma_start(
                            pt[:].rearrange("e (g d) -> e g d", g=GPC),
                            emb_t[:][:, p, 128 * sh:128 * (sh + 1), :].rearrange(
                                "g s d -> s g d"
                            ),
                        )
                        half = acc[:, 1024 * sh:1024 * (sh + 1)]
                        if p == 0:
                            nc.vector.tensor_copy(half, pt[:])
                        else:
                            nc.vector.tensor_tensor(half, half, pt[:], op=ADD)

                def build_table(src_sb_list, cc_in):
                    """src_sb_list: 2 tiles [128 h-half, 512 i] (transposed);
                    transpose to node-major [512,256] and DMA into cc_in."""
                    for ib in range(4):
                        hs_sb = wpool.tile([128, 256], f32, name=f"hs_sb{ib}")
                        for hh in range(2):
                            tp = pspool.tile([128, 128], f32, name="tp", tag="tp")
                            nc.tensor.transpose(
                                tp[:], src_sb_list[hh][:, 128 * ib:128 * (ib + 1)],
                                ident[:],
                            )
                            nc.vector.tensor_copy(
                                hs_sb[:, 128 * hh:128 * (hh + 1)], tp[:]
                            )
                        nc.sync.dma_start(cc_in[128 * ib:128 * (ib + 1), :], hs_sb[:])

                # ---- phase 1b: nxT = (dis/P * onehot-gather), h1sT = W1^T @ nxT
                nxT_sb = [wpool.tile([128, 512], f32, name=f"nxT{dh}") for dh in range(2)]
                for g in range(GPC):
                    for dh in range(2):
                        ps = ppool.tile([128, 128], f32, name="nxps", tag="mm")
                        for sh in range(2):
                            nc.tensor.matmul(
                                ps[:],
                                lhsT=acc[:, 1024 * sh + 256 * g + 128 * dh:
                                         1024 * sh + 256 * g + 128 * (dh + 1)],
                                rhs=ohnx_sb[:, (2 * g + sh) * 128:(2 * g + sh + 1) * 128],
                                start=(sh == 0), stop=(sh == 1),
                            )
                        nc.vector.tensor_copy(
                            nxT_sb[dh][:, 128 * g:128 * (g + 1)], ps[:]
                        )
                h1sT_sb = [wpool.tile([128, 512], f32, name=f"h1sT{hh}") for hh in range(2)]
                for hh in range(2):
                    ps = ppool.tile([128, 512], f32, name="h1ps", tag="mm")
                    for dh in range(2):
                        nc.tensor.matmul(
                            ps[:],
                            lhsT=w1_sb[:, dh * 256 + 128 * hh:dh * 256 + 128 * (hh + 1)],
                            rhs=nxT_sb[dh][:],
                            start=(dh == 0), stop=(dh == 1),
                        )
                    nc.vector.tensor_copy(h1sT_sb[hh][:], ps[:])
                build_table(h1sT_sb, cc1_in)

            # adjacency load: ordered after the phase-1 stream on both rings so
            # it doesn't steal HBM bandwidth from the critical stream
            for q in range(2):
                eng = nc.sync if q == 0 else nc.scalar
                a_dma = eng.dma_start(
                    adj_sb[:, 8192 * q:8192 * (q + 1)],
                    adj_t[:][:, 8192 * q:8192 * (q + 1)],
                )
                if loop_r is None and last_stream is not None:
                    _add_dep_helper(a_dma.ins, last_stream.ins, sync=False)

            nc.gpsimd.collective_compute(
                "AllGather", mybir.AluOpType.bypass, replica_groups=RG,
                ins=[cc1_in[:].opt()], outs=[table1[:].opt()],
            )

            def aggregate_T(table):
                """dense-adjacency aggregation on PE.
                Returns 2 PSUM tiles aggT[hh] = [128 h, 512 v] (transposed)."""
                # node-major hs table -> SBUF as [up, (ut, d)], 4 chunked DMAs
                hstab = hpool.tile([128, NT * 256], f32, name="hstab")
                for q in range(4):
                    eng = nc.sync if q % 2 == 0 else nc.scalar
                    eng.dma_start(
                        hstab[:, 2048 * q:2048 * (q + 1)].rearrange(
                            "p (ut d) -> p ut d", ut=8),
                        table[:][1024 * q:1024 * (q + 1), :].rearrange(
                            "(ut up) d -> up ut d", up=128),
                    )
                aggs = [ppool.tile([128, 512], f32, name=f"aggT{hh}", tag="mm")
                        for hh in range(2)]
                for ut in range(NT):
                    for hh in range(2):
                        nc.tensor.matmul(
                            aggs[hh][:],
                            lhsT=hstab[:, 256 * ut + 128 * hh:256 * ut + 128 * (hh + 1)],
                            rhs=adj_sb[:, NPC * ut:NPC * (ut + 1)],
                            start=(ut == 0), stop=(ut == NT - 1),
                        )
                return aggs

            with seg():
                # ---- layer 1 (transposed) aggregation ----
                aggT1 = aggregate_T(table1)
            with seg():
                # ----  x1s = dis*relu(dis*agg + b1) ----
                x1s = [wpool.tile([128, 512], f32, name=f"x1s{hh}") for hh in range(2)]
                for hh in range(2):
                    nc.vector.tensor_tensor(x1s[hh][:], aggT1[hh][:], disTr_sb[:], op=MUL)
                    nc.scalar.activation(x1s[hh][:], x1s[hh][:], AF.Relu,
                                         bias=b1c_sb[:, hh:hh + 1])
                    nc.vector.tensor_tensor(x1s[hh][:], x1s[hh][:], disTr_sb[:], op=MUL)
                h2sT_sb = [wpool.tile([128, 512], f32, name=f"h2sT{hh}") for hh in range(2)]
                for hh in range(2):
                    ps = ppool.tile([128, 512], f32, name="h2ps", tag="mm")
                    for h1h in range(2):
                        nc.tensor.matmul(
                            ps[:],
                            lhsT=w2_sb[:, h1h * 256 + 128 * hh:h1h * 256 + 128 * (hh + 1)],
                            rhs=x1s[h1h][:],
                            start=(h1h == 0), stop=(h1h == 1),
                        )
                    nc.vector.tensor_copy(h2sT_sb[hh][:], ps[:])
                build_table(h2sT_sb, cc2_in)

            nc.gpsimd.collective_compute(
                "AllGather", mybir.AluOpType.bypass, replica_groups=RG,
                ins=[cc2_in[:].opt()], outs=[table2[:].opt()],
            )

            with seg():
                # ---- layer 2 aggregation ----
                aggT2 = aggregate_T(table2)
            with seg():
                # ---- dis-weighted mean pool ----
                out_sb = wpool.tile([GPC, 256], f32, name="out_sb")
                for hh in range(2):
                    pm = wpool.tile([128, 512], f32, name=f"pm{hh}")
                    nc.vector.tensor_tensor(pm[:], aggT2[hh][:], pwTr_sb[:], op=MUL)
                    pr = wpool.tile([128, GPC], f32, name=f"pr{hh}")
                    nc.vector.tensor_reduce(
                        pr[:], pm[:].rearrange("h (g v) -> h g v", g=GPC),
                        axis=mybir.AxisListType.X, op=ADD,
                    )
                    nc.vector.tensor_tensor(
                        pr[:], pr[:], b2c_sb[:, hh:hh + 1].to_broadcast([128, GPC]),
                        op=ADD,
                    )
                    tp = pspool.tile([GPC, 128], f32, name="ptp", tag="tp")
                    nc.tensor.transpose(tp[:], pr[:], ident[:])
                    nc.vector.tensor_copy(out_sb[:, 128 * hh:128 * (hh + 1)], tp[:])
                nc.sync.dma_start(out_t[:], out_sb[:])

    nc.compile()
    return nc


def _get_program(loop_r=None, loop_segs=(0, 1, 2)):
    key = ("nc", loop_r, tuple(loop_segs))
    if key not in _CACHE:
        _CACHE[key] = _build_program(loop_r, loop_segs)
    return _CACHE[key]


# --------------------------------------------------------------------------
# entry point
# --------------------------------------------------------------------------
def build_in_maps(drug_graph_embedding, global_ids, edge_index, W1, b1, W2, b2):
    emb = np.ascontiguousarray(np.asarray(drug_graph_embedding), dtype=np.float32)
    W1 = np.asarray(W1, dtype=np.float32)
    W2 = np.asarray(W2, dtype=np.float32)
    b1 = np.asarray(b1, dtype=np.float32)
    b2 = np.asarray(b2, dtype=np.float32)

    prep = _host_prep(global_ids, edge_index)
    w1h = np.ascontiguousarray(
        W1.reshape(2, 128, 256).transpose(1, 0, 2).reshape(128, 512))
    w2h = np.ascontiguousarray(
        W2.reshape(2, 128, 256).transpose(1, 0, 2).reshape(128, 512))
    b1c = np.ascontiguousarray(b1.reshape(2, 128).T)
    b2c = np.ascontiguousarray(b2.reshape(2, 128).T)

    in_maps = []
    for c in range(M):
        pc = prep[c]
        in_maps.append({
            "emb": np.ascontiguousarray(emb[GPC * c:GPC * (c + 1)]),
            "ohnx": pc["ohnx"],
            "w1": w1h, "w2": w2h,
            "adj": pc["adj"],
            "disTr": pc["disTr"], "pwTr": pc["pwTr"],
            "b1c": b1c, "b2c": b2c,
        })
    return in_maps


def kernel(drug_graph_embedding, global_ids, edge_index, W1, b1, W2, b2):
    in_maps = build_in_maps(drug_graph_embedding, global_ids, edge_index,
                            W1, b1, W2, b2)
    lr = os.environ.get("BASS_KERNEL_LOOP_R")
    nc = _get_program(int(lr) if lr else None)

    if os.environ.get("BASS_KERNEL_SIM", "0") == "1":
        from concourse.bass_interp import MultiCoreSim
        sim = MultiCoreSim(nc, num_cores=M)
        for c in range(M):
            core = sim.cores[c]
            for k, v in in_maps[c].items():
                core.tensor(k)[:] = v
        sim.simulate(check_with_hw=False)
        outs = [np.array(sim.cores[c].tensor("out")) for c in range(M)]
    else:
        from concourse import bass_utils
        res = bass_utils.run_bass_kernel_spmd(
            nc, in_maps, core_ids=list(range(M)),
            trace=os.environ.get("BASS_KERNEL_TRACE", "0") == "1",
        )
        _CACHE["last_results"] = res
        outs = [res.results[c]["out"] for c in range(M)]

    return np.concatenate([o.reshape(GPC, H) for o in outs], axis=0)



# revision 9
# speedup vs baseline: 1.9837x; 1.9837x over previous
"""Trainium2 Bass kernel for nn_DrugGraphEmbedding (2-layer GCN over drug graphs).

Strategy: data-parallel over the batch axis across 8 NeuronCores (4 graphs per
core), fp16 end-to-end on the wide paths (inputs are ~N(0,1): fp16 range is
never an issue and its 11-bit mantissa keeps rel-err ~1e-3 << 2e-2 gate).

Per core:
  phase 1: stream the [4,16,256,256] embedding shard from HBM in a host
           pre-transposed fp16 layout [s, p, (sh g d)] so every DMA descriptor
           is a 4KB contiguous run (the fp32 baseline's 1KB descriptors were
           descriptor-rate limited, not bandwidth limited).  3 DMA rings
           (sync + scalar HWDGE and gpsimd SWDGE).  Vector engine accumulates
           the pathway sum in fp16 (2x DVE mode), fully hidden under the DMA.
  phase 1b: gather-by-matmul of node features (gid one-hots pre-scaled with
           dis/P on the host), W1 matmul -> node-major fp16 h1 table shard
           -> AllGather (fp16 halves the collective bytes).
  aggregation: dense-adjacency matmul on the tensor engine in fp16
           (1 cycle/row vs fp32's 4).  A [4096, 512] per core is 0/1 with
           edge multiplicity (exact in fp16), self-loops included.  The GCN
           symmetric norm factors into per-node dis[] scalings.
  layer 2 chains in the transposed layout through W2, then a dis-weighted
           pooling reduction produces the per-graph mean in fp32.
"""

import os
import numpy as np

# ---- problem constants (hardcoded per contest rules) ----
B, P, S, D = 32, 16, 256, 256
NG = 128
N = B * NG            # 4096 nodes
E = 65536
H = 256
M = 8                 # cores
GPC = B // M          # 4 graphs per core
NPC = GPC * NG        # 512 nodes per core
NT = N // 128         # 32 node tiles

_CACHE = {}


# --------------------------------------------------------------------------
# host-side preprocessing (sharding + index prep)
# --------------------------------------------------------------------------
def _host_prep(global_ids, edge_index):
    gid = np.asarray(global_ids).astype(np.int64)
    ei = np.asarray(edge_index).astype(np.int64)
    src = np.concatenate([ei[0], np.arange(N, dtype=np.int64)])
    dst = np.concatenate([ei[1], np.arange(N, dtype=np.int64)])
    deg = np.bincount(dst, minlength=N).astype(np.float32)
    dis = (1.0 / np.sqrt(deg)).astype(np.float32)

    per_core = []
    for c in range(M):
        lo, hi = NPC * c, NPC * (c + 1)
        m = (dst >= lo) & (dst < hi)
        es, ed = src[m], dst[m] - lo
        # dense adjacency, rhs layout [src%128, (src//128, dst_local)];
        # np.add.at accumulates duplicate edges; 0/1/2.. exact in fp16
        adj = np.zeros((128, NT, NPC), dtype=np.float32)
        np.add.at(adj, (es % 128, es // 128, ed), 1.0)
        # gid one-hot, scaled by dis/P:  [s_part, (g, sh, i)]
        ohnx = np.zeros((128, GPC * 2 * 128), dtype=np.float32)
        for g in range(GPC):
            gnodes = lo + 128 * g + np.arange(NG)
            s = gid[GPC * c + g]
            ohnx[s % 128, (2 * g + (s // 128)) * 128 + np.arange(NG)] = (
                dis[gnodes] / P
            )
        dis_c = dis[lo:hi]
        disTr = np.ascontiguousarray(np.broadcast_to(dis_c, (128, NPC)))
        pwTr = np.ascontiguousarray(disTr / NG)
        per_core.append(dict(
            adj=np.ascontiguousarray(adj.reshape(128, NT * NPC)).astype(np.float16),
            ohnx=ohnx.astype(np.float16),
            disTr32=disTr, disTr16=disTr.astype(np.float16), pwTr=pwTr,
        ))
    return per_core


# --------------------------------------------------------------------------
# the Bass program (one SPMD program for all 8 cores)
# --------------------------------------------------------------------------
def _build_program():
    import concourse.bacc as bacc
    import concourse.tile as tile
    import concourse.mybir as mybir
    from concourse.bass import _add_dep_helper
    from concourse.masks import make_identity

    f32 = mybir.dt.float32
    f16 = mybir.dt.float16
    AF = mybir.ActivationFunctionType
    ADD = mybir.AluOpType.add
    MUL = mybir.AluOpType.mult

    nc = bacc.Bacc("TRN2", target_bir_lowering=False, debug=False, num_devices=M)

    emb_t = nc.dram_tensor("emb", [128, P, 2 * GPC * D], f16, kind="ExternalInput")
    ohnx_t = nc.dram_tensor("ohnx", [128, GPC * 2 * 128], f16, kind="ExternalInput")
    w1_t = nc.dram_tensor("w1", [128, 512], f16, kind="ExternalInput")
    w2_t = nc.dram_tensor("w2", [128, 512], f16, kind="ExternalInput")
    adj_t = nc.dram_tensor("adj", [128, NT * NPC], f16, kind="ExternalInput")
    disTr32_t = nc.dram_tensor("disTr32", [128, NPC], f32, kind="ExternalInput")
    disTr16_t = nc.dram_tensor("disTr16", [128, NPC], f16, kind="ExternalInput")
    pwTr_t = nc.dram_tensor("pwTr", [128, NPC], f32, kind="ExternalInput")
    b1c_t = nc.dram_tensor("b1c", [128, 2], f32, kind="ExternalInput")
    b2c_t = nc.dram_tensor("b2c", [128, 2], f32, kind="ExternalInput")
    out_t = nc.dram_tensor("out", [GPC, H], f32, kind="ExternalOutput")

    cc1_in = nc.dram_tensor("cc1_in", [NPC, 256], f16, kind="Internal")
    cc2_in = nc.dram_tensor("cc2_in", [NPC, 256], f16, kind="Internal")
    table1 = nc.dram_tensor("table1", [N, 256], f16, kind="Internal",
                            addr_space="Shared")
    table2 = nc.dram_tensor("table2", [N, 256], f16, kind="Internal",
                            addr_space="Shared")
    RG = [list(range(M))]

    with tile.TileContext(nc) as tc:
        with (
            tc.tile_pool(name="const", bufs=1) as cpool,
            tc.tile_pool(name="stream", bufs=6) as spool,
            tc.tile_pool(name="work", bufs=1) as wpool,
            tc.tile_pool(name="hst", bufs=1) as hpool,
            tc.tile_pool(name="psum", bufs=4, space="PSUM") as ppool,
            tc.tile_pool(name="psmall", bufs=2, space="PSUM") as pspool,
        ):
            rings = [nc.sync, nc.scalar, nc.gpsimd]

            # ---- constants in ----
            ohnx_sb = cpool.tile([128, GPC * 2 * 128], f16, name="ohnx_sb")
            w1_sb = cpool.tile([128, 512], f16, name="w1_sb")
            w2_sb = cpool.tile([128, 512], f16, name="w2_sb")
            adj_sb = cpool.tile([128, NT * NPC], f16, name="adj_sb")
            disTr32_sb = cpool.tile([128, NPC], f32, name="disTr32_sb")
            disTr16_sb = cpool.tile([128, NPC], f16, name="disTr16_sb")
            pwTr_sb = cpool.tile([128, NPC], f32, name="pwTr_sb")
            b1c_sb = cpool.tile([128, 2], f32, name="b1c_sb")
            b2c_sb = cpool.tile([128, 2], f32, name="b2c_sb")
            ident16 = cpool.tile([128, 128], f16, name="ident16")
            ident32 = cpool.tile([128, 128], f32, name="ident32")

            nc.sync.dma_start(ohnx_sb[:], ohnx_t[:])
            nc.sync.dma_start(w1_sb[:], w1_t[:])
            nc.sync.dma_start(w2_sb[:], w2_t[:])
            nc.scalar.dma_start(disTr32_sb[:], disTr32_t[:])
            nc.scalar.dma_start(disTr16_sb[:], disTr16_t[:])
            nc.scalar.dma_start(pwTr_sb[:], pwTr_t[:])
            nc.scalar.dma_start(b1c_sb[:], b1c_t[:])
            nc.scalar.dma_start(b2c_sb[:], b2c_t[:])
            make_identity(nc, ident16[:])
            make_identity(nc, ident32[:])

            # ---- phase 1: pathway-sum accumulate  acc[s, (sh g d)] fp16 ----
            acc = cpool.tile([128, 2048], f16, name="acc")
            last_stream = [None, None, None]
            for p in range(P):
                pt = spool.tile([128, 2048], f16, name="pt")
                eng = rings[p % 3]
                last_stream[p % 3] = eng.dma_start(pt[:], emb_t[:][:, p, :])
                if p == 0:
                    nc.vector.tensor_copy(acc[:], pt[:])
                else:
                    nc.vector.tensor_tensor(acc[:], acc[:], pt[:], op=ADD)

            # adjacency load: ordered after the phase-1 stream on each ring so
            # it doesn't steal HBM bandwidth from the critical stream
            for q in range(4):
                eng = rings[q % 3]
                a_dma = eng.dma_start(
                    adj_sb[:, 4096 * q:4096 * (q + 1)],
                    adj_t[:][:, 4096 * q:4096 * (q + 1)],
                )
                if last_stream[q % 3] is not None:
                    _add_dep_helper(a_dma.ins, last_stream[q % 3].ins, sync=False)

            # ---- phase 1b: per-graph gather + W1, node-major fp16 table ----
            # nxT[dh] = acc-slice^T @ onehot  (contraction over s)
            for g in range(GPC):
                nxT_sb = wpool.tile([128, 256], f16, name=f"nxT_sb{g}")
                for dh in range(2):
                    ps = ppool.tile([128, 128], f32, name="nxps", tag="mm")
                    for sh in range(2):
                        nc.tensor.matmul(
                            ps[:],
                            lhsT=acc[:, 1024 * sh + 256 * g + 128 * dh:
                                     1024 * sh + 256 * g + 128 * (dh + 1)],
                            rhs=ohnx_sb[:, (2 * g + sh) * 128:(2 * g + sh + 1) * 128],
                            start=(sh == 0), stop=(sh == 1),
                        )
                    nc.vector.tensor_copy(nxT_sb[:, 128 * dh:128 * (dh + 1)], ps[:])
                # h1[g] node-major: lhsT = nxT (free dim = node), rhs = W1
                hps = ppool.tile([128, 256], f32, name="h1ps", tag="mm")
                for dh in range(2):
                    nc.tensor.matmul(
                        hps[:],
                        lhsT=nxT_sb[:, 128 * dh:128 * (dh + 1)],
                        rhs=w1_sb[:, 256 * dh:256 * (dh + 1)],
                        start=(dh == 0), stop=(dh == 1),
                    )
                h1_sb = wpool.tile([128, 256], f16, name="h1_sb", tag="h1sb", bufs=2)
                nc.vector.tensor_copy(h1_sb[:], hps[:])
                nc.sync.dma_start(cc1_in[128 * g:128 * (g + 1), :], h1_sb[:])

            nc.gpsimd.collective_compute(
                "AllGather", mybir.AluOpType.bypass, replica_groups=RG,
                ins=[cc1_in[:].opt()], outs=[table1[:].opt()],
            )

            def aggregate_T(table, layer):
                """dense-adjacency aggregation on PE (fp16 operands).
                Returns 2 PSUM tiles aggT[hh] = [128 h, 512 v] fp32."""
                hstab = hpool.tile([128, NT * 256], f16, name="hstab",
                                   tag="hstab", bufs=2)
                for q in range(4):
                    eng = rings[q % 2]
                    eng.dma_start(
                        hstab[:, 2048 * q:2048 * (q + 1)].rearrange(
                            "p (ut d) -> p ut d", ut=8),
                        table[:][1024 * q:1024 * (q + 1), :].rearrange(
                            "(ut up) d -> up ut d", up=128),
                    )
                aggs = [ppool.tile([128, 512], f32, name=f"aggT{hh}_{layer}",
                                   tag="mm")
                        for hh in range(2)]
                for ut in range(NT):
                    for hh in range(2):
                        nc.tensor.matmul(
                            aggs[hh][:],
                            lhsT=hstab[:, 256 * ut + 128 * hh:256 * ut + 128 * (hh + 1)],
                            rhs=adj_sb[:, NPC * ut:NPC * (ut + 1)],
                            start=(ut == 0), stop=(ut == NT - 1),
                        )
                return aggs

            # ---- layer 1 aggregation (transposed) ----
            aggT1 = aggregate_T(table1, 1)

            # ----  x1s = dis*relu(dis*agg + b1), fp16 ----
            x1s = [wpool.tile([128, 512], f16, name=f"x1s{hh}") for hh in range(2)]
            for hh in range(2):
                x1t = wpool.tile([128, 512], f32, name="x1t", tag="x1t", bufs=2)
                nc.vector.tensor_tensor(x1t[:], aggT1[hh][:], disTr32_sb[:], op=MUL)
                x1r = wpool.tile([128, 512], f16, name="x1r", tag="x1r", bufs=2)
                nc.scalar.activation(x1r[:], x1t[:], AF.Relu,
                                     bias=b1c_sb[:, hh:hh + 1])
                nc.vector.tensor_tensor(x1s[hh][:], x1r[:], disTr16_sb[:], op=MUL)

            # ---- h2sT = W2^T @ x1s  (transposed), then node-major table ----
            h2s_sb = [wpool.tile([128, 512], f16, name=f"h2s{hh}") for hh in range(2)]
            for hh in range(2):
                ps = ppool.tile([128, 512], f32, name="h2ps", tag="mm")
                for h1h in range(2):
                    nc.tensor.matmul(
                        ps[:],
                        lhsT=w2_sb[:, h1h * 256 + 128 * hh:h1h * 256 + 128 * (hh + 1)],
                        rhs=x1s[h1h][:],
                        start=(h1h == 0), stop=(h1h == 1),
                    )
                nc.vector.tensor_copy(h2s_sb[hh][:], ps[:])
            # transpose [h, v] -> node-major [v, h] and write cc2_in
            for ib in range(4):
                hs_sb = wpool.tile([128, 256], f16, name="hs_sb", tag="hs", bufs=2)
                for hh in range(2):
                    tp = pspool.tile([128, 128], f16, name="tp16", tag="tp",
                                     padded_shape=[128, 1024])
                    nc.tensor.transpose(
                        tp[:], h2s_sb[hh][:, 128 * ib:128 * (ib + 1)], ident16[:],
                    )
                    nc.vector.tensor_copy(hs_sb[:, 128 * hh:128 * (hh + 1)], tp[:])
                nc.sync.dma_start(cc2_in[128 * ib:128 * (ib + 1), :], hs_sb[:])

            nc.gpsimd.collective_compute(
                "AllGather", mybir.AluOpType.bypass, replica_groups=RG,
                ins=[cc2_in[:].opt()], outs=[table2[:].opt()],
            )

            # ---- layer 2 aggregation ----
            aggT2 = aggregate_T(table2, 2)

            # ---- dis-weighted mean pool (fp32) ----
            out_sb = wpool.tile([GPC, 256], f32, name="out_sb")
            for hh in range(2):
                pm = wpool.tile([128, 512], f32, name="pm", tag="pm", bufs=2)
                nc.vector.tensor_tensor(pm[:], aggT2[hh][:], pwTr_sb[:], op=MUL)
                pr = wpool.tile([128, GPC], f32, name="pr", tag="pr", bufs=2)
                nc.vector.tensor_reduce(
                    pr[:], pm[:].rearrange("h (g v) -> h g v", g=GPC),
                    axis=mybir.AxisListType.X, op=ADD,
                )
                nc.vector.tensor_tensor(
                    pr[:], pr[:], b2c_sb[:, hh:hh + 1].to_broadcast([128, GPC]),
                    op=ADD,
                )
                tp = pspool.tile([GPC, 128], f32, name="ptp", tag="ptp",
                                 padded_shape=[GPC, 512])
                nc.tensor.transpose(tp[:], pr[:], ident32[:])
                nc.vector.tensor_copy(out_sb[:, 128 * hh:128 * (hh + 1)], tp[:])
            nc.sync.dma_start(out_t[:], out_sb[:])

    nc.compile()
    return nc


def _get_program():
    if "nc" not in _CACHE:
        _CACHE["nc"] = _build_program()
    return _CACHE["nc"]


# --------------------------------------------------------------------------
# entry point
# --------------------------------------------------------------------------
def build_in_maps(drug_graph_embedding, global_ids, edge_index, W1, b1, W2, b2):
    emb = np.asarray(drug_graph_embedding, dtype=np.float32)
    W1 = np.asarray(W1, dtype=np.float32)
    W2 = np.asarray(W2, dtype=np.float32)
    b1 = np.asarray(b1, dtype=np.float32)
    b2 = np.asarray(b2, dtype=np.float32)

    prep = _host_prep(global_ids, edge_index)
    # emb host relayout: [B,P,S,D] -> per-core [s128, p, (sh g d)] fp16 so the
    # device stream is 4KB-contiguous per partition row per p-chunk
    emb16 = emb.astype(np.float16).reshape(B, P, 2, 128, D)
    w1h = np.ascontiguousarray(
        W1.reshape(2, 128, 256).transpose(1, 0, 2).reshape(128, 512)
    ).astype(np.float16)
    w2h = np.ascontiguousarray(
        W2.reshape(2, 128, 256).transpose(1, 0, 2).reshape(128, 512)
    ).astype(np.float16)
    b1c = np.ascontiguousarray(b1.reshape(2, 128).T)
    b2c = np.ascontiguousarray(b2.reshape(2, 128).T)

    in_maps = []
    for c in range(M):
        pc = prep[c]
        # [g, p, sh, s, d] -> [s, p, sh, g, d]
        embc = np.ascontiguousarray(
            emb16[GPC * c:GPC * (c + 1)].transpose(3, 1, 2, 0, 4)
        ).reshape(128, P, 2 * GPC * D)
        in_maps.append({
            "emb": embc,
            "ohnx": pc["ohnx"],
            "w1": w1h, "w2": w2h,
            "adj": pc["adj"],
            "disTr32": pc["disTr32"], "disTr16": pc["disTr16"],
            "pwTr": pc["pwTr"],
            "b1c": b1c, "b2c": b2c,
        })
    return in_maps


def kernel(drug_graph_embedding, global_ids, edge_index, W1, b1, W2, b2):
    in_maps = build_in_maps(drug_graph_embedding, global_ids, edge_index,
                            W1, b1, W2, b2)
    nc = _get_program()

    if os.environ.get("BASS_KERNEL_SIM", "0") == "1":
        from concourse.bass_interp import MultiCoreSim
        sim = MultiCoreSim(nc, num_cores=M)
        for c in range(M):
            core = sim.cores[c]
            for k, v in in_maps[c].items():
                core.tensor(k)[:] = v
        sim.simulate(check_with_hw=False)
        outs = [np.array(sim.cores[c].tensor("out")) for c in range(M)]
    else:
        from concourse import bass_utils
        res = bass_utils.run_bass_kernel_spmd(
            nc, in_maps, core_ids=list(range(M)),
            trace=os.environ.get("BASS_KERNEL_TRACE", "0") == "1",
        )
        _CACHE["last_results"] = res
        outs = [res.results[c]["out"] for c in range(M)]

    return np.concatenate([o.reshape(GPC, H) for o in outs], axis=0)
